# revision 5
# baseline (speedup 1.0000x reference)
"""Trainium2 Bass kernel for nn_BaselineDNN (embedding pooling + MLP), v2.

Reference computation (B=2048, L=200, V=50000, D=300, H=128, C=20):
    emb = emb_table[x]                       # [B, L, D] gather
    s   = sum(emb, axis=1); mx = max(emb, axis=1)
    rep = concat([s / len^2, mx], -1)        # [B, 600]
    h   = relu(rep @ W_new.T + b_new)        # [B, 128]
    out = h @ W3.T + b3                      # [B, 20]

Sharding: data-parallel over batch across 8 cores (256 rows/core).

Design (vs the 608us v1: 400 single-column indirect DMAs + fp32
identity-matmul sum; this version: ~156us):
  - host compacts the table per (core, token-half): unique ids of 256x100
    tokens (<= 25600, fits int16) -> bf16 rows at 768B stride; token
    indices become int16 ranks.
  - device gathers via InstDMAGatherAnt (mlp gpsimd library), built
    directly so elem_size=600B rides a 768B stride (bass's %256 assert is
    a transpose-path restriction). 1280 idx per instruction, 40 total,
    spread round-robin over 4 SWDGE queues: the Pool NX pipelines
    instructions across queue cpu-pairs, 4x-ing descriptor generation
    (~8ns/idx per pair is the serial floor). single_packet=False is
    required (single-packet mode exceeds the 64-desc packet ceiling and
    wedges the device); small chunks keep 2-3 instructions' descriptors
    (81/engine) inside the 256-slot rings so transfers pipeline.
  - max: bf16 tensor_max pair-tree on DVE (2x_1P mode) with a ping-pong
    running max of 5 token-slots per chunk; final fold emits fp32-free.
  - sum: bf16 identity-matmul accumulation into fp32 PSUM (3x cheaper
    than fp32 matmuls; exact up to the bf16 table rounding).
  - MLP tail in bf16, PSUM->SBUF copies on the scalar engine.
  End-to-end rel err ~3.4e-3 vs the 2e-2 gate (bf16 rounding).
"""

import numpy as np
import ml_dtypes

import concourse.bacc as bacc
import concourse.bass as bass
import concourse.mybir as mybir
import concourse.tile as tile
from concourse import library_config
from concourse.bass_utils import run_bass_kernel_spmd

F32 = mybir.dt.float32
BF16 = mybir.dt.bfloat16
I16 = mybir.dt.int16


def _dma_gather_rows(
    nc, out_ap, in_ap, idxs_ap, num_idxs, elem_size, elem_step, queue_num=0
):
    """dma_gather (non-transpose, HBM source) with elem_size not tied to the
    256B granule. Only the row STRIDE is encoded as stride/256 in the
    descriptor; the transfer length is arbitrary (>=512B for SDMA line
    rate). bass.dma_gather asserts elem_size%256==0 ("transpose
    restriction") even for the non-transpose path, so we build the
    instruction here with the same lowering calls bass uses.

    out_ap: [128, n, elem_size] SBUF; in_ap: [rows, elem_size] DRAM view
    with ap[0] stride == elem_step; idxs int16 [128, num_idxs/16].
    """
    from concourse import ap_utils

    eng = nc.gpsimd
    eng._assert_queue_num(queue_num)
    assert idxs_ap.dtype == I16
    assert in_ap.dtype == out_ap.dtype
    elem_size_bytes = elem_size * mybir.dt.size(in_ap.dtype)
    stride_bytes = elem_step * mybir.dt.size(in_ap.dtype)
    assert elem_size_bytes >= 512, "sub-512B transfers hit SDMA RMW"
    assert stride_bytes % 256 == 0
    stride_bytes_256 = stride_bytes // 256
    assert stride_bytes_256 < 256
    assert ap_utils.ap_is_contiguous(in_ap.ap[1:])
    assert ap_utils.ap_is_contiguous(out_ap.ap[1:])
    assert ap_utils.ap_is_contiguous(idxs_ap.ap[1:])
    assert num_idxs % 128 == 0
    assert in_ap.ap[-1][1] == out_ap.ap[-1][1] == elem_size
    assert out_ap.ap[0][1] * out_ap.ap[1][1] == num_idxs
    assert in_ap.ap[0][0] == elem_step

    _in_ap = eng.lower_ap_dma(in_ap, for_custom_bir_dma=True)
    _idxs_ap = eng.lower_ap(idxs_ap)
    _out_ap = eng.lower_ap(out_ap)
    return eng.add_instruction(
        mybir.InstDMAGatherAnt(
            name=nc.get_next_instruction_name(),
            ins=[
                *_in_ap,
                _idxs_ap,
                eng.lower_val_access(eng.to_reg(num_idxs)),
            ],
            outs=[_out_ap],
            transpose=False,
            num_idxs=num_idxs,
            elem_size=elem_size,
            stride_bytes_256=stride_bytes_256,
            gen_mode=0,
            single_packet=False,
            queue_num=queue_num,
            sbuf_tokens_per_rank=0,
            sbuf_free_dim_per_rank=0,
            sbuf_free_dim_pad_per_rank=0,
            sbuf_byte_offset=0,
        )
    )

B, L, V, D, H, C = 2048, 200, 50000, 300, 128, 20
NCORES = 8
BL = B // NCORES          # 256 rows per core
P = 128                   # partitions
G = BL // P               # 2 groups of 128 rows
KD = 5                    # d-chunks of 128 for the 600-dim rep (640 padded)
DPAD = KD * P             # 640
DEPAD = 384               # emb row padded to 384 bf16 = 768B (256B multiple)
HALF = L // 2             # 100 tokens per compaction half
UMAX = BL * HALF          # per-half unique hard bound: 256 rows x 100 tokens
T = 10                    # max token-columns per dma_gather chunk
CSZS = (10,) * 10                 # per-half chunk sizes (sums to HALF)
CHUNKS_PER_G = 2 * len(CSZS)      # 20 chunks per group
NRUN = 5                  # running-max slots from the pair tree
# processing order: group-major, then half, then chunk
SCHED = []
_off = 0
for _g in range(2):
    for _h in range(2):
        _col = 0
        for _csz in CSZS:
            SCHED.append((_g, _h, _col, _csz, _off))
            _off += 8 * _csz          # idx block width: 128*csz/16
            _col += _csz
IDXTOT = _off             # 3200 int16 per partition


def build_program(gather_bufs: int = 8, nq: int = 1):
    nc = bacc.Bacc(
        "TRN2", target_bir_lowering=False, debug=False, num_swdge_queues=nq
    )

    tabs = [
        nc.dram_tensor(f"tab{h}", [UMAX, DEPAD], BF16, kind="ExternalInput").ap()
        for h in range(2)
    ]
    idx = nc.dram_tensor("idx", [P, IDXTOT], I16, kind="ExternalInput").ap()
    invl = nc.dram_tensor("invl", [P, G], F32, kind="ExternalInput").ap()
    wnewt = nc.dram_tensor("wnewt", [KD, P, H], BF16, kind="ExternalInput").ap()
    w3t = nc.dram_tensor("w3t", [H, C], BF16, kind="ExternalInput").ap()
    bnew = nc.dram_tensor("bnew", [H, 1], F32, kind="ExternalInput").ap()
    b3 = nc.dram_tensor("b3", [C, 1], F32, kind="ExternalInput").ap()
    iden = nc.dram_tensor("iden", [P, P], F32, kind="ExternalInput").ap()
    idenb = nc.dram_tensor("idenb", [P, P], BF16, kind="ExternalInput").ap()
    out = nc.dram_tensor("out", [C, BL], F32, kind="ExternalOutput").ap()

    with tile.TileContext(nc) as tc:
        nc.gpsimd.load_library(library_config.mlp)
        with (
            tc.tile_pool(name="const", bufs=1) as const_pool,
            tc.tile_pool(name="gath", bufs=gather_bufs) as gather_pool,
            tc.tile_pool(name="work", bufs=2) as work_pool,
            tc.tile_pool(name="psum", bufs=2, space="PSUM") as psum_pool,
        ):
            idx_sb = const_pool.tile([P, IDXTOT], I16)
            nc.sync.dma_start(out=idx_sb[:], in_=idx[:])
            invl_sb = const_pool.tile([P, G], F32)
            nc.sync.dma_start(out=invl_sb[:], in_=invl[:])
            iden_sb = const_pool.tile([P, P], F32)
            nc.sync.dma_start(out=iden_sb[:], in_=iden[:])
            idenb_sb = const_pool.tile([P, P], BF16)
            nc.sync.dma_start(out=idenb_sb[:], in_=idenb[:])
            wnewt_sb = const_pool.tile([P, KD, H], BF16)
            nc.sync.dma_start(out=wnewt_sb[:], in_=wnewt[:].transpose([1, 0, 2]))
            w3t_sb = const_pool.tile([H, C], BF16)
            nc.sync.dma_start(out=w3t_sb[:], in_=w3t[:])
            bnew_sb = const_pool.tile([H, 1], F32)
            nc.sync.dma_start(out=bnew_sb[:], in_=bnew[:])
            b3_sb = const_pool.tile([C, 1], F32)
            nc.sync.dma_start(out=b3_sb[:], in_=b3[:])

            # [d-part, k-chunk, batch(2 groups)] transposed rep for the MLP
            rep_t = const_pool.tile([P, KD, BL], BF16)

            psum_s = [
                psum_pool.tile([P, D], F32, tag=f"psum_s{g}", name=f"psum_s{g}")
                for g in range(G)
            ]
            runbuf = [
                [
                    work_pool.tile(
                        [P, NRUN, D], BF16, tag=f"run{g}{i}", name=f"run{g}{i}"
                    )
                    for i in range(2)
                ]
                for g in range(G)
            ]
            tmp2 = const_pool.tile([P, NRUN, D], BF16)
            cnt = [0, 0]
            for qi, (g, h, col, csz, off) in enumerate(SCHED):
                k = cnt[g]
                cnt[g] += 1
                gt = gather_pool.tile([P, T, D], BF16, tag="gt")
                # 600B transfers from 768B-strided rows; one packet per
                # descriptor (single_packet would blow the 64-desc packet
                # ceiling and wedge the device).
                _dma_gather_rows(
                    nc,
                    gt[:, 0:csz, :],
                    tabs[h][:, 0:D],
                    idx_sb[:, off : off + 8 * csz],
                    P * csz,
                    D,
                    DEPAD,
                    queue_num=qi % nq,
                )
                # max pair-tree: 10 -> 5, then running max over the 5 slots
                lvl1_out = runbuf[g][0] if k == 0 else tmp2
                nc.vector.tensor_max(
                    lvl1_out[:], gt[:, 0:10:2, 0:D], gt[:, 1:10:2, 0:D]
                )
                if k > 0:
                    nc.vector.tensor_max(
                        runbuf[g][k % 2][:], runbuf[g][(k + 1) % 2][:], tmp2[:]
                    )
                # sum: accumulate each token column into PSUM (bf16 PE)
                for j in range(csz):
                    nc.tensor.matmul(
                        out=psum_s[g][:],
                        lhsT=idenb_sb[:],
                        rhs=gt[:, j, 0:D],
                        start=(k == 0 and j == 0),
                        stop=(k == CHUNKS_PER_G - 1 and j == csz - 1),
                    )

                if k != CHUNKS_PER_G - 1:
                    continue
                # group finished: fold the 5 running slots -> 1 (bf16 rep)
                run = runbuf[g][(CHUNKS_PER_G - 1) % 2]
                rep = work_pool.tile([P, DPAD], BF16, tag="rep")
                nc.scalar.memzero(rep[:, 2 * D : DPAD])
                fa = work_pool.tile([P, 2, D], BF16, tag="fa")
                nc.vector.tensor_max(fa[:], run[:, 0:2, :], run[:, 2:4, :])
                fb = work_pool.tile([P, 1, D], BF16, tag="fb")
                nc.vector.tensor_max(fb[:], fa[:, 0:1, :], fa[:, 1:2, :])
                nc.vector.tensor_max(rep[:, D : 2 * D], fb[:, 0, :], run[:, 4, :])

                # mean_bug = s / len^2
                nc.vector.tensor_scalar_mul(
                    rep[:, 0:D], psum_s[g][:], invl_sb[:, g : g + 1]
                )
                # transpose rep -> rep_t[:, k, g*128:(g+1)*128] (bf16 PE
                # transposes; PSUM->SBUF copies on the idle scalar engine so
                # they don't queue behind DVE chunk work)
                for kk in range(KD):
                    pt = psum_pool.tile([P, P], BF16, tag="pt")
                    nc.tensor.transpose(
                        out=pt[:],
                        in_=rep[:, kk * P : (kk + 1) * P],
                        identity=idenb_sb[:],
                    )
                    nc.scalar.copy(rep_t[:, kk, g * P : (g + 1) * P], pt[:])

            # h = relu(rep @ W_new.T + b_new): out[h, b]
            psum_h = psum_pool.tile([P, BL], F32, tag="psum_h", bufs=1)
            for kk in range(KD):
                nc.tensor.matmul(
                    out=psum_h[:],
                    lhsT=wnewt_sb[:, kk, :],
                    rhs=rep_t[:, kk, :],
                    start=(kk == 0),
                    stop=(kk == KD - 1),
                )
            h_sb = work_pool.tile([P, BL], BF16)
            nc.scalar.activation(
                h_sb[:],
                psum_h[:],
                mybir.ActivationFunctionType.Relu,
                bias=bnew_sb[:],
                scale=1.0,
            )
            # logits = h @ W3.T + b3: out[c, b]
            psum_l = psum_pool.tile([C, BL], F32, tag="psum_l", bufs=1)
            nc.tensor.matmul(
                out=psum_l[:], lhsT=w3t_sb[:], rhs=h_sb[:], start=True, stop=True
            )
            lo_sb = work_pool.tile([C, BL], F32)
            nc.vector.tensor_scalar_add(lo_sb[:], psum_l[:], b3_sb[:])
            nc.sync.dma_start(out=out[:], in_=lo_sb[:])

    nc.compile()
    return nc


def make_in_maps(x, lengths, emb_table, W_new, b_new, W3, b3):
    emb_np = np.asarray(emb_table, dtype=np.float32)
    x_np = np.asarray(x).astype(np.int64)
    len_f = np.asarray(lengths).astype(np.float32)
    inv_len2 = (1.0 / (len_f * len_f)).astype(np.float32)

    wnewt_pad = np.zeros((DPAD, H), dtype=ml_dtypes.bfloat16)
    wnewt_pad[: 2 * D, :] = np.asarray(W_new, dtype=np.float32).T
    wnewt_np = np.ascontiguousarray(wnewt_pad.reshape(KD, P, H))
    w3t_np = np.ascontiguousarray(
        np.asarray(W3, dtype=np.float32).T.astype(ml_dtypes.bfloat16)
    )
    bnew_np = np.asarray(b_new, dtype=np.float32).reshape(H, 1)
    b3_np = np.asarray(b3, dtype=np.float32).reshape(C, 1)
    iden_np = np.eye(P, dtype=np.float32)
    idenb_np = np.eye(P, dtype=ml_dtypes.bfloat16)

    in_maps = []
    for c in range(NCORES):
        xl = x_np[c * BL : (c + 1) * BL]            # [256, 200]
        il = inv_len2[c * BL : (c + 1) * BL]        # [256]
        invl_np = np.ascontiguousarray(il.reshape(G, P).T)

        tabs = []
        ranks = []
        for h in range(2):
            xh = xl[:, h * HALF : (h + 1) * HALF]   # [256, 100]
            uids, r = np.unique(xh, return_inverse=True)
            assert uids.size <= UMAX
            tab = np.zeros((UMAX, DEPAD), dtype=ml_dtypes.bfloat16)
            tab[: uids.size, :D] = emb_np[uids].astype(ml_dtypes.bfloat16)
            tabs.append(tab)
            ranks.append(r.reshape(xh.shape).astype(np.int16))

        # idx blocks in SCHED order, each [16, 8*csz] tiled to 128 partitions;
        # position i of a chunk = (token jj = i//128, partition p = i%128)
        blocks = []
        for g, h, col, csz, off in SCHED:
            r = ranks[h][g * P : (g + 1) * P, col : col + csz]
            seq = r.T.reshape(-1)                   # [128*csz] i = jj*128 + p
            blocks.append(seq.reshape(8 * csz, 16).T)
        idx_np = np.tile(np.concatenate(blocks, axis=1), (8, 1))
        idx_np = np.ascontiguousarray(idx_np)

        in_maps.append(
            {
                "tab0": tabs[0],
                "tab1": tabs[1],
                "idx": idx_np,
                "invl": invl_np,
                "wnewt": wnewt_np,
                "w3t": w3t_np,
                "bnew": bnew_np,
                "b3": b3_np,
                "iden": iden_np,
                "idenb": idenb_np,
            }
        )
    return in_maps


def run(inputs, trace=False, gather_bufs=8, tmpdir=None, nq=4):
    nc = build_program(gather_bufs=gather_bufs, nq=nq)
    in_maps = make_in_maps(**inputs)
    res = run_bass_kernel_spmd(
        nc, in_maps, core_ids=list(range(NCORES)), trace=trace, tmpdir=tmpdir
    )
    outs = [res.results[c]["out"].T for c in range(NCORES)]  # each [256, 20]
    full = np.concatenate(outs, axis=0).astype(np.float32)
    return full, res


def kernel(**inputs) -> np.ndarray:
    full, _ = run(inputs, trace=False)
    return full


# revision 7
# speedup vs baseline: 1.0537x; 1.0537x over previous
"""Trainium2 Bass kernel for nn_BaselineDNN (embedding pooling + MLP), v2.

Reference computation (B=2048, L=200, V=50000, D=300, H=128, C=20):
    emb = emb_table[x]                       # [B, L, D] gather
    s   = sum(emb, axis=1); mx = max(emb, axis=1)
    rep = concat([s / len^2, mx], -1)        # [B, 600]
    h   = relu(rep @ W_new.T + b_new)        # [B, 128]
    out = h @ W3.T + b3                      # [B, 20]

Sharding: data-parallel over batch across 8 cores (256 rows/core).

Design (vs the 608us v1: 400 single-column indirect DMAs + fp32
identity-matmul sum; this version: ~152us):
  - host compacts the table per (core, token-half): unique ids of 256x100
    tokens (<= 25600, fits int16) -> bf16 rows at 768B stride; token
    indices become int16 ranks.
  - device gathers via InstDMAGatherAnt (mlp gpsimd library), built
    directly so elem_size=600B rides a 768B stride (bass's %256 assert is
    a transpose-path restriction). 1280 idx per instruction, 40 total,
    spread round-robin over 4 SWDGE queues: the Pool NX pipelines
    instructions across queue cpu-pairs, 4x-ing descriptor generation
    (~8ns/idx per pair is the serial floor). single_packet=False is
    required (single-packet mode exceeds the 64-desc packet ceiling and
    wedges the device); small chunks keep 2-3 instructions' descriptors
    (81/engine) inside the 256-slot rings so transfers pipeline.
  - max: bf16 tensor_max pair-tree on DVE (2x_1P mode) with a ping-pong
    running max of 5 token-slots per chunk; final fold emits fp32-free.
  - sum: bf16 identity-matmul accumulation into fp32 PSUM (3x cheaper
    than fp32 matmuls; exact up to the bf16 table rounding).
  - MLP tail in bf16 and split per group (g0's transposes+MLP+output DMA
    overlap g1's gather stream); PSUM->SBUF copies on the scalar engine.
  End-to-end rel err ~3.4e-3 vs the 2e-2 gate (bf16 rounding).
"""

import numpy as np
import ml_dtypes

import concourse.bacc as bacc
import concourse.bass as bass
import concourse.mybir as mybir
import concourse.tile as tile
from concourse import library_config
from concourse.bass_utils import run_bass_kernel_spmd

F32 = mybir.dt.float32
BF16 = mybir.dt.bfloat16
I16 = mybir.dt.int16


def _dma_gather_rows(
    nc, out_ap, in_ap, idxs_ap, num_idxs, elem_size, elem_step, queue_num=0
):
    """dma_gather (non-transpose, HBM source) with elem_size not tied to the
    256B granule. Only the row STRIDE is encoded as stride/256 in the
    descriptor; the transfer length is arbitrary (>=512B for SDMA line
    rate). bass.dma_gather asserts elem_size%256==0 ("transpose
    restriction") even for the non-transpose path, so we build the
    instruction here with the same lowering calls bass uses.

    out_ap: [128, n, elem_size] SBUF; in_ap: [rows, elem_size] DRAM view
    with ap[0] stride == elem_step; idxs int16 [128, num_idxs/16].
    """
    from concourse import ap_utils

    eng = nc.gpsimd
    eng._assert_queue_num(queue_num)
    assert idxs_ap.dtype == I16
    assert in_ap.dtype == out_ap.dtype
    elem_size_bytes = elem_size * mybir.dt.size(in_ap.dtype)
    stride_bytes = elem_step * mybir.dt.size(in_ap.dtype)
    assert elem_size_bytes >= 512, "sub-512B transfers hit SDMA RMW"
    assert stride_bytes % 256 == 0
    stride_bytes_256 = stride_bytes // 256
    assert stride_bytes_256 < 256
    assert ap_utils.ap_is_contiguous(in_ap.ap[1:])
    assert ap_utils.ap_is_contiguous(out_ap.ap[1:])
    assert ap_utils.ap_is_contiguous(idxs_ap.ap[1:])
    assert num_idxs % 128 == 0
    assert in_ap.ap[-1][1] == out_ap.ap[-1][1] == elem_size
    assert out_ap.ap[0][1] * out_ap.ap[1][1] == num_idxs
    assert in_ap.ap[0][0] == elem_step

    _in_ap = eng.lower_ap_dma(in_ap, for_custom_bir_dma=True)
    _idxs_ap = eng.lower_ap(idxs_ap)
    _out_ap = eng.lower_ap(out_ap)
    return eng.add_instruction(
        mybir.InstDMAGatherAnt(
            name=nc.get_next_instruction_name(),
            ins=[
                *_in_ap,
                _idxs_ap,
                eng.lower_val_access(eng.to_reg(num_idxs)),
            ],
            outs=[_out_ap],
            transpose=False,
            num_idxs=num_idxs,
            elem_size=elem_size,
            stride_bytes_256=stride_bytes_256,
            gen_mode=0,
            single_packet=False,
            queue_num=queue_num,
            sbuf_tokens_per_rank=0,
            sbuf_free_dim_per_rank=0,
            sbuf_free_dim_pad_per_rank=0,
            sbuf_byte_offset=0,
        )
    )

B, L, V, D, H, C = 2048, 200, 50000, 300, 128, 20
NCORES = 8
BL = B // NCORES          # 256 rows per core
P = 128                   # partitions
G = BL // P               # 2 groups of 128 rows
KD = 5                    # d-chunks of 128 for the 600-dim rep (640 padded)
DPAD = KD * P             # 640
DEPAD = 384               # emb row padded to 384 bf16 = 768B (256B multiple)
HALF = L // 2             # 100 tokens per compaction half
UMAX = BL * HALF          # per-half unique hard bound: 256 rows x 100 tokens
T = 10                    # max token-columns per dma_gather chunk
CSZS = (10,) * 10                 # per-half chunk sizes (sums to HALF)
CHUNKS_PER_G = 2 * len(CSZS)      # 20 chunks per group
NRUN = 5                  # running-max slots from the pair tree
# processing order: group-major, then half, then chunk
SCHED = []
_off = 0
for _g in range(2):
    for _h in range(2):
        _col = 0
        for _csz in CSZS:
            SCHED.append((_g, _h, _col, _csz, _off))
            _off += 8 * _csz          # idx block width: 128*csz/16
            _col += _csz
IDXTOT = _off             # 3200 int16 per partition


def build_program(gather_bufs: int = 8, nq: int = 1):
    nc = bacc.Bacc(
        "TRN2", target_bir_lowering=False, debug=False, num_swdge_queues=nq
    )

    tabs = [
        nc.dram_tensor(f"tab{h}", [UMAX, DEPAD], BF16, kind="ExternalInput").ap()
        for h in range(2)
    ]
    idx = nc.dram_tensor("idx", [P, IDXTOT], I16, kind="ExternalInput").ap()
    invl = nc.dram_tensor("invl", [P, G], F32, kind="ExternalInput").ap()
    wnewt = nc.dram_tensor("wnewt", [KD, P, H], BF16, kind="ExternalInput").ap()
    w3t = nc.dram_tensor("w3t", [H, C], BF16, kind="ExternalInput").ap()
    bnew = nc.dram_tensor("bnew", [H, 1], F32, kind="ExternalInput").ap()
    b3 = nc.dram_tensor("b3", [C, 1], F32, kind="ExternalInput").ap()
    iden = nc.dram_tensor("iden", [P, P], F32, kind="ExternalInput").ap()
    idenb = nc.dram_tensor("idenb", [P, P], BF16, kind="ExternalInput").ap()
    out = nc.dram_tensor("out", [C, BL], F32, kind="ExternalOutput").ap()

    with tile.TileContext(nc) as tc:
        nc.gpsimd.load_library(library_config.mlp)
        with (
            tc.tile_pool(name="const", bufs=1) as const_pool,
            tc.tile_pool(name="gath", bufs=gather_bufs) as gather_pool,
            tc.tile_pool(name="work", bufs=2) as work_pool,
            tc.tile_pool(name="psum", bufs=2, space="PSUM") as psum_pool,
        ):
            idx_sb = const_pool.tile([P, IDXTOT], I16)
            nc.sync.dma_start(out=idx_sb[:], in_=idx[:])
            invl_sb = const_pool.tile([P, G], F32)
            nc.sync.dma_start(out=invl_sb[:], in_=invl[:])
            iden_sb = const_pool.tile([P, P], F32)
            nc.sync.dma_start(out=iden_sb[:], in_=iden[:])
            idenb_sb = const_pool.tile([P, P], BF16)
            nc.sync.dma_start(out=idenb_sb[:], in_=idenb[:])
            wnewt_sb = const_pool.tile([P, KD, H], BF16)
            nc.sync.dma_start(out=wnewt_sb[:], in_=wnewt[:].transpose([1, 0, 2]))
            w3t_sb = const_pool.tile([H, C], BF16)
            nc.sync.dma_start(out=w3t_sb[:], in_=w3t[:])
            bnew_sb = const_pool.tile([H, 1], F32)
            nc.sync.dma_start(out=bnew_sb[:], in_=bnew[:])
            b3_sb = const_pool.tile([C, 1], F32)
            nc.sync.dma_start(out=b3_sb[:], in_=b3[:])

            # [d-part, k-chunk, batch(2 groups)] transposed rep for the MLP
            rep_t = const_pool.tile([P, KD, BL], BF16)

            psum_s = [
                psum_pool.tile([P, D], F32, tag=f"psum_s{g}", name=f"psum_s{g}")
                for g in range(G)
            ]
            runbuf = [
                [
                    work_pool.tile(
                        [P, NRUN, D], BF16, tag=f"run{g}{i}", name=f"run{g}{i}"
                    )
                    for i in range(2)
                ]
                for g in range(G)
            ]
            tmp2 = const_pool.tile([P, NRUN, D], BF16)
            cnt = [0, 0]
            for qi, (g, h, col, csz, off) in enumerate(SCHED):
                k = cnt[g]
                cnt[g] += 1
                gt = gather_pool.tile([P, T, D], BF16, tag="gt")
                # 600B transfers from 768B-strided rows; one packet per
                # descriptor (single_packet would blow the 64-desc packet
                # ceiling and wedge the device).
                _dma_gather_rows(
                    nc,
                    gt[:, 0:csz, :],
                    tabs[h][:, 0:D],
                    idx_sb[:, off : off + 8 * csz],
                    P * csz,
                    D,
                    DEPAD,
                    queue_num=qi % nq,
                )
                # max pair-tree: 10 -> 5, then running max over the 5 slots
                lvl1_out = runbuf[g][0] if k == 0 else tmp2
                nc.vector.tensor_max(
                    lvl1_out[:], gt[:, 0:10:2, 0:D], gt[:, 1:10:2, 0:D]
                )
                if k > 0:
                    nc.vector.tensor_max(
                        runbuf[g][k % 2][:], runbuf[g][(k + 1) % 2][:], tmp2[:]
                    )
                # sum: accumulate each token column into PSUM (bf16 PE)
                for j in range(csz):
                    nc.tensor.matmul(
                        out=psum_s[g][:],
                        lhsT=idenb_sb[:],
                        rhs=gt[:, j, 0:D],
                        start=(k == 0 and j == 0),
                        stop=(k == CHUNKS_PER_G - 1 and j == csz - 1),
                    )

                if k != CHUNKS_PER_G - 1:
                    continue
                # group finished: fold the 5 running slots -> 1 (bf16 rep)
                run = runbuf[g][(CHUNKS_PER_G - 1) % 2]
                rep = work_pool.tile([P, DPAD], BF16, tag="rep")
                nc.scalar.memzero(rep[:, 2 * D : DPAD])
                fa = work_pool.tile([P, 2, D], BF16, tag="fa")
                nc.vector.tensor_max(fa[:], run[:, 0:2, :], run[:, 2:4, :])
                fb = work_pool.tile([P, 1, D], BF16, tag="fb")
                nc.vector.tensor_max(fb[:], fa[:, 0:1, :], fa[:, 1:2, :])
                nc.vector.tensor_max(rep[:, D : 2 * D], fb[:, 0, :], run[:, 4, :])

                # mean_bug = s / len^2
                nc.vector.tensor_scalar_mul(
                    rep[:, 0:D], psum_s[g][:], invl_sb[:, g : g + 1]
                )
                # transpose rep -> rep_t[:, k, g*128:(g+1)*128] (bf16 PE
                # transposes; PSUM->SBUF copies on the idle scalar engine so
                # they don't queue behind DVE chunk work)
                for kk in range(KD):
                    pt = psum_pool.tile([P, P], BF16, tag="pt")
                    nc.tensor.transpose(
                        out=pt[:],
                        in_=rep[:, kk * P : (kk + 1) * P],
                        identity=idenb_sb[:],
                    )
                    nc.scalar.copy(rep_t[:, kk, g * P : (g + 1) * P], pt[:])

                # per-group MLP + output: g0's whole tail overlaps g1's
                # gather stream; only g1's chain remains at the end
                gs = slice(g * P, (g + 1) * P)
                psum_h = psum_pool.tile([P, P], F32, tag="psum_h",
                                        name=f"psum_h{g}", bufs=1)
                for kk in range(KD):
                    nc.tensor.matmul(
                        out=psum_h[:],
                        lhsT=wnewt_sb[:, kk, :],
                        rhs=rep_t[:, kk, gs],
                        start=(kk == 0),
                        stop=(kk == KD - 1),
                    )
                h_sb = work_pool.tile([P, P], BF16, tag="h_sb")
                nc.scalar.activation(
                    h_sb[:],
                    psum_h[:],
                    mybir.ActivationFunctionType.Relu,
                    bias=bnew_sb[:],
                    scale=1.0,
                )
                psum_l = psum_pool.tile([C, P], F32, tag="psum_l",
                                        name=f"psum_l{g}", bufs=1)
                nc.tensor.matmul(
                    out=psum_l[:], lhsT=w3t_sb[:], rhs=h_sb[:],
                    start=True, stop=True,
                )
                lo_sb = work_pool.tile([C, P], F32, tag="lo_sb")
                nc.vector.tensor_scalar_add(lo_sb[:], psum_l[:], b3_sb[:])
                nc.sync.dma_start(out=out[:, gs], in_=lo_sb[:])

    nc.compile()
    return nc


def make_in_maps(x, lengths, emb_table, W_new, b_new, W3, b3):
    emb_np = np.asarray(emb_table, dtype=np.float32)
    x_np = np.asarray(x).astype(np.int64)
    len_f = np.asarray(lengths).astype(np.float32)
    inv_len2 = (1.0 / (len_f * len_f)).astype(np.float32)

    wnewt_pad = np.zeros((DPAD, H), dtype=ml_dtypes.bfloat16)
    wnewt_pad[: 2 * D, :] = np.asarray(W_new, dtype=np.float32).T
    wnewt_np = np.ascontiguousarray(wnewt_pad.reshape(KD, P, H))
    w3t_np = np.ascontiguousarray(
        np.asarray(W3, dtype=np.float32).T.astype(ml_dtypes.bfloat16)
    )
    bnew_np = np.asarray(b_new, dtype=np.float32).reshape(H, 1)
    b3_np = np.asarray(b3, dtype=np.float32).reshape(C, 1)
    iden_np = np.eye(P, dtype=np.float32)
    idenb_np = np.eye(P, dtype=ml_dtypes.bfloat16)

    in_maps = []
    for c in range(NCORES):
        xl = x_np[c * BL : (c + 1) * BL]            # [256, 200]
        il = inv_len2[c * BL : (c + 1) * BL]        # [256]
        invl_np = np.ascontiguousarray(il.reshape(G, P).T)

        tabs = []
        ranks = []
        for h in range(2):
            xh = xl[:, h * HALF : (h + 1) * HALF]   # [256, 100]
            uids, r = np.unique(xh, return_inverse=True)
            assert uids.size <= UMAX
            tab = np.zeros((UMAX, DEPAD), dtype=ml_dtypes.bfloat16)
            tab[: uids.size, :D] = emb_np[uids].astype(ml_dtypes.bfloat16)
            tabs.append(tab)
            ranks.append(r.reshape(xh.shape).astype(np.int16))

        # idx blocks in SCHED order, each [16, 8*csz] tiled to 128 partitions;
        # position i of a chunk = (token jj = i//128, partition p = i%128)
        blocks = []
        for g, h, col, csz, off in SCHED:
            r = ranks[h][g * P : (g + 1) * P, col : col + csz]
            seq = r.T.reshape(-1)                   # [128*csz] i = jj*128 + p
            blocks.append(seq.reshape(8 * csz, 16).T)
        idx_np = np.tile(np.concatenate(blocks, axis=1), (8, 1))
        idx_np = np.ascontiguousarray(idx_np)

        in_maps.append(
            {
                "tab0": tabs[0],
                "tab1": tabs[1],
                "idx": idx_np,
                "invl": invl_np,
                "wnewt": wnewt_np,
                "w3t": w3t_np,
                "bnew": bnew_np,
                "b3": b3_np,
                "iden": iden_np,
                "idenb": idenb_np,
            }
        )
    return in_maps


def run(inputs, trace=False, gather_bufs=8, tmpdir=None, nq=4):
    nc = build_program(gather_bufs=gather_bufs, nq=nq)
    in_maps = make_in_maps(**inputs)
    res = run_bass_kernel_spmd(
        nc, in_maps, core_ids=list(range(NCORES)), trace=trace, tmpdir=tmpdir
    )
    outs = [res.results[c]["out"].T for c in range(NCORES)]  # each [256, 20]
    full = np.concatenate(outs, axis=0).astype(np.float32)
    return full, res


def kernel(**inputs) -> np.ndarray:
    full, _ = run(inputs, trace=False)
    return full
